# revision 1
# baseline (speedup 1.0000x reference)
"""Causal multi-head attention (B=2, S=2048, D=1024, 16 heads x 64) on 8
Trainium2 NeuronCores.

Sharding: tensor-parallel over heads — 2 heads per core. Each core gets the
full (pre-transposed, bf16-cast) activations and its 2 heads' weights,
computes q/k/v projections, causal flash-style attention, and a partial
output projection; the host sums the 8 partial outputs and adds b_O.

Device algorithm per core (all matmuls bf16 with fp32 PSUM accumulate):
  - QKV:   qT/kT/vT [128=2*64 headdims, 4096 tok] = W.T @ xT, accumulated
           over 8 contraction chunks of 128.
  - V is re-laid-out to [pos, headdim] via DVE 32x32 block transposes, with
    a ones-column appended so the attention-value matmul also produces the
    softmax denominator for free.
  - scores are computed transposed (key position on partitions) so softmax's
    sum folds into the AV matmul; exp runs on the scalar engine straight out
    of PSUM (no max subtraction needed: scores are O(1) by construction).
  - causal mask is a 0/1 bf16 multiply on the 4 diagonal key-tiles only.
  - 1/sum computed as exp(-ln(sum)) on the scalar engine, broadcast across
    partitions with a rank-2 matmul, applied while evacuating z.
  - out projection contracts both heads (128 partitions) in one matmul.
"""

import functools

import numpy as np
import ml_dtypes

import concourse.bass as bass
import concourse.tile as tile
import concourse.mybir as mybir
from concourse.bass_utils import run_bass_kernel_spmd

# ---------------------------------------------------------------- wait fix
# This container's walrus accepts at most ONE sync-wait per instruction
# (two for EventSemaphore); Tile emits several. Hoist the excess onto NoOps
# inserted just before the over-subscribed instruction on the same engine.
import json as _json

_WAIT_CAP = {"EventSemaphore": 2}


def _split_waits(doc):
    n = [0]

    def fix_block(block):
        insts = block.get("instructions")
        if not isinstance(insts, list):
            return
        out = []
        for inst in insts:
            si = inst.get("sync_info")
            waits = si.get("on_wait") if si else None
            cap = _WAIT_CAP.get(inst.get("opcode"), 1)
            if waits and len(waits) > cap:
                for w in waits[cap:]:
                    n[0] += 1
                    out.append(
                        {
                            "name": f"WSPL-{n[0]}",
                            "opcode": "NoOp",
                            "engine": inst["engine"],
                            "ins": [],
                            "outs": [],
                            "sync_info": {"on_wait": [w], "on_update": []},
                        }
                    )
                si["on_wait"] = waits[:cap]
            out.append(inst)
        block["instructions"] = out

    def walk(o):
        if isinstance(o, dict):
            if "instructions" in o:
                fix_block(o)
            for v in o.values():
                walk(v)
        elif isinstance(o, list):
            for v in o:
                walk(v)

    walk(doc)
    return doc


_waitfix_done = False


def _install_waitfix():
    global _waitfix_done
    if _waitfix_done:
        return
    _waitfix_done = True
    orig = bass.Bass.to_json_bytes

    def to_json_bytes(self, *a, **kw):
        doc = _json.loads(orig(self, *a, **kw))
        return _json.dumps(_split_waits(doc)).encode()

    bass.Bass.to_json_bytes = to_json_bytes


# ---------------------------------------------------------------- constants
B, S, D = 2, 2048, 1024
NHEAD, HDIM = 16, 64
T = B * S  # 4096 tokens
NCORES = 8
HPC = NHEAD // NCORES  # 2 heads per core
SCALE = 1.0 / 8.0  # 1/sqrt(HDIM)

bf16 = mybir.dt.bfloat16
f32 = mybir.dt.float32
AF = mybir.ActivationFunctionType

NDC = D // 128  # 8 contraction chunks
NPT = T // 512  # 8 pos-tiles of 512
NKT = S // 128  # 16 key tiles per batch
NQB = S // 512  # 4 query blocks per batch


def _build_nc():
    nc = bass.Bass()
    xT = nc.dram_tensor("xT", [D, T], bf16, kind="ExternalInput")
    wqkv = nc.dram_tensor("wqkv", [D, 3 * 128], bf16, kind="ExternalInput")
    bqkv = nc.dram_tensor("bqkv", [128, 3], f32, kind="ExternalInput")
    wo = nc.dram_tensor("wo", [128, D], bf16, kind="ExternalInput")
    masks = nc.dram_tensor("masks", [128, 4, 1024], bf16, kind="ExternalInput")
    ones1 = nc.dram_tensor("ones1", [2, 128], bf16, kind="ExternalInput")
    outp = nc.dram_tensor("outp", [T, D], bf16, kind="ExternalOutput")

    with tile.TileContext(nc) as tc:
        with (
            tc.tile_pool(name="const", bufs=1) as const,
            tc.tile_pool(name="attn", bufs=8) as attnp,
            tc.tile_pool(name="obuf", bufs=4) as obufp,
            tc.tile_pool(name="small", bufs=4) as small,
            tc.tile_pool(name="psum", bufs=2, space="PSUM") as psum,
        ):
            # ---- constant loads (weights + first x chunks first: QKV's
            # critical path; masks/wo are needed much later)
            w_sb = const.tile([128, NDC, 3 * 128], bf16)
            wq4 = wqkv[:].rearrange("(a p) c -> p a c", p=128)
            xt_sb = const.tile([128, NDC, T], bf16)
            xTr = xT[:].rearrange("(a p) m -> p a m", p=128)
            # V-group weights, then the first x chunk: exactly what the
            # first QKV matmul needs; everything else follows
            nc.sync.dma_start(w_sb[:, :, 256:384], wq4[:, :, 256:384])
            nc.sync.dma_start(xt_sb[:, :, 0:512], xTr[:, :, 0:512])
            for g in (0, 1):
                nc.sync.dma_start(
                    w_sb[:, :, 128 * g : 128 * g + 128],
                    wq4[:, :, 128 * g : 128 * g + 128],
                )
            bias_sb = const.tile([128, 3], f32)
            nc.sync.dma_start(bias_sb[:], bqkv[:])

            # x chunks in QKV-consumption order (batch-alternating blocks)
            for i, k in enumerate((1, 2, 3, 4, 5, 6, 7)):
                sl = slice(512 * k, 512 * (k + 1))
                nc.sync.dma_start(xt_sb[:, :, sl], xTr[:, :, sl])
                if i == 0:
                    ee_sb = const.tile([2, 128], bf16)
                    nc.sync.dma_start(ee_sb[:], ones1[:])
                if i == 1:
                    mask_sb = const.tile([128, 4, 1024], bf16)
                    nc.sync.dma_start(mask_sb[:], masks[:])
                if i == 3:
                    wo_sb = const.tile([128, D], bf16)
                    nc.sync.dma_start(wo_sb[:], wo[:])

            qT = const.tile([128, T], bf16)
            kT = const.tile([128, T], bf16)
            vT = const.tile([128, T], bf16)
            zT = const.tile([128, T], bf16)
            qkvT = (qT, kT, vT)

            v_sb = []
            for h in range(HPC):
                v = const.tile([128, T // 128, 65], bf16, name=f"v_sb{h}")
                nc.gpsimd.memset(v[:, :, 64], 1.0)
                v_sb.append(v)

            # ---- phase A: QKV projections (V group first so the V
            # re-layout can start early), with V transposes interleaved
            # per 1024-token block.
            vt4 = vT[:].rearrange("p (t x i) -> p t x i", x=4, i=32)

            def emit_qkv(pp):
                for g in (2, 0, 1):
                    ps = psum.tile([128, 1024], f32, tag="sc", bufs=3)
                    for half in range(2):
                        pt = 2 * pp + half
                        dst = ps[:, 512 * half : 512 * half + 512]
                        for di in range(NDC):
                            nc.tensor.matmul(
                                dst,
                                w_sb[:, di, 128 * g : 128 * g + 128],
                                xt_sb[:, di, 512 * pt : 512 * pt + 512],
                                start=(di == 0),
                                stop=(di == NDC - 1),
                            )
                    # bias-add + cast on ACT: the scalar engine is idle
                    # during QKV while the vector engine runs V transposes
                    nc.scalar.activation(
                        qkvT[g][:, 1024 * pp : 1024 * pp + 1024],
                        ps[:],
                        AF.Identity,
                        bias=bias_sb[:, g : g + 1],
                        scale=1.0,
                    )
                # V block [1024*pp, 1024*(pp+1)) -> [pos, headdim] tiles
                ts = slice(8 * pp, 8 * pp + 8)
                for h in range(HPC):
                    for al in range(4):
                        for bb in range(2):
                            nc.vector.transpose(
                                v_sb[h][
                                    32 * al : 32 * al + 32, ts, 32 * bb : 32 * bb + 32
                                ],
                                vt4[
                                    64 * h + 32 * bb : 64 * h + 32 * bb + 32, ts, al, :
                                ],
                            )

            # ---- phase C: attention per (batch, 512-query-block);
            # batches interleaved so one batch's softmax-normalize tail
            # overlaps the other batch's score/AV work. Big blocks first
            # so the pipeline fills. Each block's output projection is
            # emitted one unit LATE: the in-order PE then always has
            # ready matmuls to chew on while the previous block's
            # normalize chain completes.
            def emit_outproj(qb, b, tail=False):
                for qx in range(4):
                    qt = NKT * b + 4 * qb + qx
                    op = psum.tile([128, 1024], f32, tag="sc", bufs=3, name="op")
                    for dh in range(2):
                        nc.tensor.matmul(
                            op[:, 512 * dh : 512 * dh + 512],
                            zT[:, 128 * qt : 128 * qt + 128],
                            wo_sb[:, 512 * dh : 512 * dh + 512],
                            start=True,
                            stop=True,
                        )
                    ob = obufp.tile([128, 1024], bf16, name="ob")
                    if tail and qx % 2 == 1:
                        # drain: no exps left, so the idle scalar engine
                        # shares the PSUM evacuation load with DVE
                        nc.scalar.copy(ob[:], op[:])
                    else:
                        nc.vector.tensor_copy(ob[:], op[:])
                    nc.sync.dma_start(outp[128 * qt : 128 * qt + 128, :], ob[:])

            def norm_stage_a(st):
                # 1/sum = exp(-ln(sum)); both heads' sums were DMA-staged
                # onto partitions {0,1} of one tile, so one ln and one exp
                # cover both heads
                q0, zsU, rsin, rs2 = st
                lnS = small.tile([2, 512], f32, tag="lnS")
                nc.scalar.activation(lnS[:], rsin[:], AF.Ln, scale=1.0)
                nc.scalar.activation(rs2[:], lnS[:], AF.Exp, scale=-1.0)

            def norm_stage_b(st):
                # broadcast both heads' reciprocals to 128 partitions with
                # one K=2 matmul against the 0/1 selector matrix ee_sb,
                # then normalize the staged z into zT
                q0, zsU, rsin, rs2 = st
                rbP = psum.tile([128, 512], f32, tag="sc", bufs=3, name="rbP")
                nc.tensor.matmul(rbP[:], ee_sb[:], rs2[:], start=True, stop=True)
                for h in range(HPC):
                    nc.vector.tensor_mul(
                        zT[64 * h : 64 * h + 64, q0 : q0 + 512],
                        zsU[h][0:64, :],
                        rbP[64 * h : 64 * h + 64, :],
                    )

            st = {"norm_a": None, "norm_b": None, "uidx": 0}
            out_queue = []  # (uidx, qb, b); emitted two units late

            def emit_unit(qb, b):
                uidx = st["uidx"]
                st["uidx"] += 1
                norm_a = st["norm_a"]
                norm_b = st["norm_b"]
                nkt = 4 * (qb + 1)  # causal: key tiles 0..4qb+3
                q0 = S * b + 512 * qb
                zp = [
                    psum.tile([65, 512], f32, tag="z", bufs=2, name=f"zp{h}")
                    for h in range(HPC)
                ]
                for kt in range(nkt):
                    gk = NKT * b + kt
                    sp = psum.tile([128, 1024], f32, tag="sc", bufs=3)
                    for h in range(HPC):
                        nc.tensor.matmul(
                            sp[:, 512 * h : 512 * h + 512],
                            kT[64 * h : 64 * h + 64, 128 * gk : 128 * gk + 128],
                            qT[64 * h : 64 * h + 64, q0 : q0 + 512],
                            start=True,
                            stop=True,
                        )
                    at = attnp.tile([128, 1024], bf16)
                    nc.scalar.activation(at[:], sp[:], AF.Exp, scale=SCALE)
                    if kt >= 4 * qb:
                        # causal mask on the diagonal tiles
                        j = kt - 4 * qb
                        nc.vector.tensor_mul(at[:], at[:], mask_sb[:, j, :])
                    for h in range(HPC):
                        nc.tensor.matmul(
                            zp[h][:],
                            v_sb[h][:, gk, :],
                            at[:, 512 * h : 512 * h + 512],
                            start=(kt == 0),
                            stop=(kt == nkt - 1),
                        )
                    if kt == 1 and norm_a is not None:
                        norm_stage_a(norm_a)
                        norm_b = norm_a
                        norm_a = None
                    if kt == min(4, nkt - 2) and norm_b is not None:
                        norm_stage_b(norm_b)
                        norm_b = None
                    if (
                        kt == min(5, nkt - 1)
                        and out_queue
                        and out_queue[0][0] <= uidx - 2
                    ):
                        _, oqb, ob_ = out_queue.pop(0)
                        emit_outproj(oqb, ob_)
                # evacuate z and its sums row to SBUF immediately so the
                # PSUM banks free up for the next query block; a small DMA
                # gathers the two sums rows onto partitions {0,1} of one
                # tile (DMA writes have no partition-alignment limits)
                zsU = [
                    small.tile([65, 512], bf16, tag=f"zsU{h}", name=f"zsU{h}")
                    for h in range(HPC)
                ]
                rsin = small.tile([2, 512], bf16, tag="rsin")
                rs2 = small.tile([2, 512], bf16, tag="rs2")
                for h in range(HPC):
                    nc.vector.tensor_copy(zsU[h][:], zp[h][:])
                    nc.sync.dma_start(rsin[h : h + 1, :], zsU[h][64:65, :])
                st["norm_a"] = (q0, zsU, rsin, rs2)
                st["norm_b"] = norm_b
                out_queue.append((uidx, qb, b))

            # ---- master schedule: QKV blocks interleaved with attention
            # units as their inputs complete (pp0=b0 tok 0-1023, pp2=b1
            # tok 2048-3071, ...), so attention's scalar/vector work
            # overlaps QKV's dense matmul stream
            for pp in range(NPT // 2):
                emit_qkv(pp)
            for qb in range(NQB):
                for b in range(B):
                    emit_unit(qb, b)

            norm_stage_a(st["norm_a"])
            norm_stage_b(st["norm_a"])
            for _, oqb, ob_ in out_queue:
                emit_outproj(oqb, ob_, tail=True)

    return nc


@functools.lru_cache(maxsize=1)
def _get_nc():
    _install_waitfix()
    return _build_nc()


def _to_bf16(a):
    return np.ascontiguousarray(np.asarray(a, dtype=np.float32)).astype(
        ml_dtypes.bfloat16
    )


def _prepare_in_maps(
    normalized_resid_pre, W_Q, W_K, W_V, W_O, b_Q, b_K, b_V, b_O
):
    x = np.asarray(normalized_resid_pre, dtype=np.float32)
    W_Q = np.asarray(W_Q, dtype=np.float32)
    W_K = np.asarray(W_K, dtype=np.float32)
    W_V = np.asarray(W_V, dtype=np.float32)
    W_O = np.asarray(W_O, dtype=np.float32)
    b_Q = np.asarray(b_Q, dtype=np.float32)
    b_K = np.asarray(b_K, dtype=np.float32)
    b_V = np.asarray(b_V, dtype=np.float32)
    b_O = np.asarray(b_O, dtype=np.float32)

    xT = _to_bf16(x.reshape(T, D).T)

    masks_np = np.zeros((128, 4, 1024), np.float32)
    kk = np.arange(128)[:, None]
    qq = np.arange(512)[None, :]
    for j in range(4):
        m = (128 * j + kk <= qq).astype(np.float32)
        masks_np[:, j, :512] = m
        masks_np[:, j, 512:] = m
    masks_np = masks_np.astype(ml_dtypes.bfloat16)

    ones_np = np.zeros((2, 128), np.float32)
    ones_np[0, :64] = 1.0
    ones_np[1, 64:] = 1.0
    ones_np = ones_np.astype(ml_dtypes.bfloat16)

    in_maps = []
    for c in range(NCORES):
        h0, h1 = HPC * c, HPC * c + 1
        wqkv_c = np.concatenate(
            [W_Q[h0], W_Q[h1], W_K[h0], W_K[h1], W_V[h0], W_V[h1]], axis=1
        )
        bqkv_c = np.stack(
            [
                np.concatenate([b_Q[h0], b_Q[h1]]),
                np.concatenate([b_K[h0], b_K[h1]]),
                np.concatenate([b_V[h0], b_V[h1]]),
            ],
            axis=1,
        ).astype(np.float32)
        wo_c = np.concatenate([W_O[h0], W_O[h1]], axis=0)
        in_maps.append(
            {
                "xT": xT,
                "wqkv": _to_bf16(wqkv_c),
                "bqkv": np.ascontiguousarray(bqkv_c),
                "wo": _to_bf16(wo_c),
                "masks": masks_np,
                "ones1": ones_np,
            }
        )
    return in_maps, b_O


def _gather(res, b_O):
    out = np.zeros((T, D), np.float32)
    for r in res.results:
        out += r["outp"].astype(np.float32)
    out += b_O[None, :]
    return out.reshape(B, S, D)


def kernel(
    normalized_resid_pre, W_Q, W_K, W_V, W_O, b_Q, b_K, b_V, b_O, **_unused
):
    in_maps, b_O = _prepare_in_maps(
        normalized_resid_pre, W_Q, W_K, W_V, W_O, b_Q, b_K, b_V, b_O
    )
    nc = _get_nc()
    res = run_bass_kernel_spmd(nc, in_maps, core_ids=list(range(NCORES)))
    return _gather(res, b_O)


def _try_install_profhook():
    """Register the axon NTFF profile hook (the container's antenv stub
    lacks axon_hooks); harmless no-op if anything is missing."""
    try:
        import sys
        import types

        if "antenv.axon_hooks" not in sys.modules:
            mod = types.ModuleType("antenv.axon_hooks")
            hook = [None]
            mod.set_axon_ntff_profile_hook = lambda h: hook.__setitem__(0, h)
            mod.get_axon_ntff_profile_hook = lambda: hook[0]
            sys.modules["antenv.axon_hooks"] = mod
            import antenv

            antenv.axon_hooks = mod
            from trn_agent_boot.trn_boot import _ntff_profile_via_ctypes

            mod.set_axon_ntff_profile_hook(
                _ntff_profile_via_ctypes("/opt/axon/libaxon_pjrt.so")
            )
            import concourse.bass_utils as bu

            bu.upload_artifacts = lambda tmpdir: f"file://{tmpdir}"
    except Exception:
        pass


def kernel_profiled(**inputs):
    """Like kernel() but with NTFF tracing; returns (out, BassKernelResults)."""
    _try_install_profhook()
    inputs = {k: v for k, v in inputs.items()}
    in_maps, b_O = _prepare_in_maps(
        inputs["normalized_resid_pre"],
        inputs["W_Q"],
        inputs["W_K"],
        inputs["W_V"],
        inputs["W_O"],
        inputs["b_Q"],
        inputs["b_K"],
        inputs["b_V"],
        inputs["b_O"],
    )
    nc = _get_nc()
    res = run_bass_kernel_spmd(
        nc, in_maps, core_ids=list(range(NCORES)), trace=True
    )
    return _gather(res, b_O), res


if __name__ == "__main__":
    rng = np.random.default_rng(0)
    inputs = {
        "normalized_resid_pre": rng.standard_normal((B, S, D)).astype(np.float32),
        "W_Q": (rng.standard_normal((NHEAD, D, HDIM)) * 0.02).astype(np.float32),
        "W_K": (rng.standard_normal((NHEAD, D, HDIM)) * 0.02).astype(np.float32),
        "W_V": (rng.standard_normal((NHEAD, D, HDIM)) * 0.02).astype(np.float32),
        "W_O": (rng.standard_normal((NHEAD, HDIM, D)) * 0.02).astype(np.float32),
        "b_Q": np.zeros((NHEAD, HDIM), np.float32),
        "b_K": np.zeros((NHEAD, HDIM), np.float32),
        "b_V": np.zeros((NHEAD, HDIM), np.float32),
        "b_O": np.zeros((D,), np.float32),
    }
    out = kernel(**inputs)
    print("out", out.shape, out.dtype, float(np.abs(out).max()))

